# revision 8
# baseline (speedup 1.0000x reference)
"""Distributed multi-head attention forward for 8 TRN2 NeuronCores.

Problem: y = proj(softmax((x Wq^T + bq)(x Wk^T + bk)^T / sqrt(hd)) (x Wv^T + bv))
  x: [4, 2048, 1024], 16 heads, head_dim 64, fp32.

Sharding: token-parallel. Core i owns global flat tokens [i*1024, (i+1)*1024)
(cores 2b, 2b+1 own batch b). Each core projects q/k/v for its own tokens,
a pairwise AllGather exchanges k/v between the two cores of each batch, then
each core computes full 16-head attention for its 1024 query tokens and the
output projection locally. Output is token-sharded; no trailing collective.

Layouts (host pre-transposes, all free):
  xT      [D, tq]    feature-major activations
  w_{q,k}T[D, D]     so qT/kT come out feature-major: qT[f, t]
  w_vT    [D, D]     v computed token-major: v[t, f]
  bias    [128, D/128] partition-major (per-partition scalars for ACT bias)
  w_projT [D, D]
  out     yT [D, tq] (host transposes back)

b_v is folded into b_proj on the host: (o + bv) Wp^T + bp = o Wp^T + (Wp bv + bp).

Softmax skips the max subtraction (scores ~N(0, 0.17), |s|max < ~4, exp safe
in fp32) so the sum of exp can ride the AV matmul as a ones-column on V:
  OT_aug[0:64, q]  = sum_k v[k, d] p[k, q]      (pre-normalized output^T)
  OT_aug[64, q]    = sum_k p[k, q]              (softmax denominator Z)
Normalization: 1/Z via DVE reciprocal (batched [16, qch]), gpsimd
partition_broadcast to [64, qch], one DVE multiply.
"""

import numpy as np

P = 128
D = 1024
NH = 16
HD = 64
SCALE = 1.0 / float(np.sqrt(HD))
NCORES = 8
TQ = 1024          # tokens per core
B, T = 4, 2048

_COMPILED = {}


def _full_cfg():
    return dict(D=D, NH=NH, TQ=TQ, n_devices=NCORES,
                replica_groups=[[0, 1], [2, 3], [4, 5], [6, 7]])


def build(cfg=None):
    """Build + compile the per-core Bass graph. Returns the compiled Bacc."""
    from concourse import bacc
    import concourse.mybir as mybir
    import concourse.tile as tile

    if cfg is None:
        cfg = _full_cfg()
    d = cfg["D"]; nh = cfg["NH"]; tq = cfg["TQ"]
    n_dev = cfg["n_devices"]; rgroups = cfg["replica_groups"]
    tk = 2 * tq                      # batch tokens for k/v (2 cores per batch)
    f32 = mybir.dt.float32
    f32r = mybir.dt.float32r
    AF = mybir.ActivationFunctionType

    nft = d // P                     # feature tiles (also contraction chunks)
    qch = min(512, tq)               # q free-dim chunk
    nqc = tq // qch
    nkt = tk // P                    # k tiles along batch tokens
    nhp = nh // 2                    # head pairs
    # k-tile groups of 3 for batched exp (3 PSUM banks per exp call)
    kgroups = []
    i = 0
    while i < nkt:
        g = min(3, nkt - i)
        kgroups.append((i, g))
        i += g

    nc = bacc.Bacc("TRN2", target_bir_lowering=False, debug=False,
                   num_devices=n_dev)

    xT = nc.dram_tensor("xT", [d, tq], f32, kind="ExternalInput")
    w_qT = nc.dram_tensor("w_qT", [d, d], f32, kind="ExternalInput")
    w_kT = nc.dram_tensor("w_kT", [d, d], f32, kind="ExternalInput")
    w_vT = nc.dram_tensor("w_vT", [d, d], f32, kind="ExternalInput")
    w_pT = nc.dram_tensor("w_pT", [d, d], f32, kind="ExternalInput")
    b_q = nc.dram_tensor("b_q", [P, nft], f32, kind="ExternalInput")
    b_k = nc.dram_tensor("b_k", [P, nft], f32, kind="ExternalInput")
    b_p = nc.dram_tensor("b_p", [P, nft], f32, kind="ExternalInput")
    ones = nc.dram_tensor("ones", [P, nkt, 1], f32, kind="ExternalInput")
    outT = nc.dram_tensor("out", [d, tq], f32, kind="ExternalOutput")

    with tile.TileContext(nc) as tc:
        with (
            tc.tile_pool(name="dram", bufs=1, space="DRAM") as dram,
            tc.tile_pool(name="persist", bufs=1) as persist,
            tc.tile_pool(name="bias", bufs=1) as biasp,
        ):
            cc_in = dram.tile([2 * tq, d], f32)
            # addr_space="Shared" is only supported for replica groups > 4
            # cores; pairwise groups must use a Local output buffer.
            cc_out = dram.tile([2 * tk, d], f32)

            # ---- persistent SBUF ----
            x_sb = persist.tile([P, nft, tq], f32r)
            nc.sync.dma_start(x_sb[:], xT.rearrange("(c p) t -> p c t", p=P).bitcast(f32r))
            q_all = persist.tile([P, nft, tq], f32r)
            ot_all = persist.tile([P, nft, tq], f32r)
            wp_sb = persist.tile([P, nft, d], f32r)
            nc.sync.dma_start(wp_sb[:], w_pT.rearrange("(c p) f -> p c f", p=P).bitcast(f32r))
            bq_sb = biasp.tile([P, nft], f32)
            nc.sync.dma_start(bq_sb[:], b_q[:])
            bk_sb = biasp.tile([P, nft], f32)
            nc.sync.dma_start(bk_sb[:], b_k[:])
            bp_sb = biasp.tile([P, nft], f32)
            nc.sync.dma_start(bp_sb[:], b_p[:])

            # ================= phase 1: k and v projections =================
            with (
                tc.tile_pool(name="wpool", bufs=2) as wpool,
                tc.tile_pool(name="kvout", bufs=3) as kvout,
                tc.tile_pool(name="psmm", bufs=4, space="PSUM") as psmm,
            ):
                # kT[f, t] = sum_d w_kT[d, f] x[t, d]   (weight-stationary)
                wk_sb = wpool.tile([P, nft, d], f32r, tag="w")
                nc.sync.dma_start(wk_sb[:], w_kT.rearrange("(c p) f -> p c f", p=P).bitcast(f32r))
                for ft in range(nft):
                    for c in range(nqc):
                        ps = psmm.tile([P, qch], f32)
                        for dc in range(nft):
                            nc.tensor.matmul(
                                ps[:],
                                wk_sb[:, dc, ft * P:(ft + 1) * P],
                                x_sb[:, dc, c * qch:(c + 1) * qch],
                                start=(dc == 0), stop=(dc == nft - 1))
                        ksb = kvout.tile([P, qch], f32, tag="kv")
                        nc.scalar.activation(ksb[:], ps[:], AF.Identity,
                                             bias=bk_sb[:, ft:ft + 1])
                        nc.sync.dma_start(
                            cc_in[ft * P:(ft + 1) * P, c * qch:(c + 1) * qch],
                            ksb[:])

                # v[t, f] = sum_d x[t, d] w_vT[d, f]    (x-stationary)
                wv_sb = wpool.tile([P, nft, d], f32r, tag="w")
                nc.sync.dma_start(wv_sb[:], w_vT.rearrange("(c p) f -> p c f", p=P).bitcast(f32r))
                fch = min(512, d)
                for tt in range(tq // P):
                    for fc in range(d // fch):
                        ps = psmm.tile([P, fch], f32)
                        for dc in range(nft):
                            nc.tensor.matmul(
                                ps[:],
                                x_sb[:, dc, tt * P:(tt + 1) * P],
                                wv_sb[:, dc, fc * fch:(fc + 1) * fch],
                                start=(dc == 0), stop=(dc == nft - 1))
                        vsb = kvout.tile([P, fch], f32, tag="kv")
                        nc.vector.tensor_copy(vsb[:], ps[:])
                        nc.sync.dma_start(
                            cc_in[tq + tt * P:tq + (tt + 1) * P,
                                  fc * fch:(fc + 1) * fch],
                            vsb[:])

                # pairwise exchange of k/v
                nc.gpsimd.collective_compute(
                    "AllGather", mybir.AluOpType.bypass,
                    replica_groups=rgroups,
                    ins=[cc_in.opt()], outs=[cc_out.opt()])

                # ---- q projection (overlaps the collective) ----
                wq_sb = wpool.tile([P, nft, d], f32r, tag="w")
                nc.sync.dma_start(wq_sb[:], w_qT.rearrange("(c p) f -> p c f", p=P).bitcast(f32r))
                for ft in range(nft):
                    for c in range(nqc):
                        ps = psmm.tile([P, qch], f32)
                        for dc in range(nft):
                            nc.tensor.matmul(
                                ps[:],
                                wq_sb[:, dc, ft * P:(ft + 1) * P],
                                x_sb[:, dc, c * qch:(c + 1) * qch],
                                start=(dc == 0), stop=(dc == nft - 1))
                        nc.scalar.activation(
                            q_all[:, ft, c * qch:(c + 1) * qch], ps[:],
                            AF.Identity, bias=bq_sb[:, ft:ft + 1])

            # ================= phase 2: attention + out projection ==========
            with (
                tc.tile_pool(name="ktp", bufs=2) as ktp,
                tc.tile_pool(name="vp", bufs=4) as vp,
                tc.tile_pool(name="pst", bufs=2, space="PSUM") as pst,
                tc.tile_pool(name="pot", bufs=1, space="PSUM") as pot,
                tc.tile_pool(name="psy", bufs=1, space="PSUM") as psy,
                tc.tile_pool(name="ptile", bufs=3) as ptile,
                tc.tile_pool(name="zpool", bufs=2) as zpool,
                tc.tile_pool(name="rzbp", bufs=2) as rzbp,
                tc.tile_pool(name="ypool", bufs=2) as ypool,
            ):
                for qc in range(nqc):
                    for hp in range(nhp):
                        # kT for head pair hp: features [hp*128, (hp+1)*128)
                        # rows 0:tq from my rank, tq:tk from the partner
                        kt = ktp.tile([P, tk], f32r)
                        nc.sync.dma_start(kt[:, 0:tq],
                                          cc_out[hp * P:(hp + 1) * P, 0:tq]
                                          .bitcast(f32r))
                        nc.sync.dma_start(kt[:, tq:tk],
                                          cc_out[2 * tq + hp * P:2 * tq + (hp + 1) * P, 0:tq]
                                          .bitcast(f32r))
                        for hh in range(2):
                            h = 2 * hp + hh
                            # v for head h, augmented with a ones column
                            vt = vp.tile([P, nkt, HD + 1], f32r, tag="v")
                            nc.sync.dma_start(
                                vt[:, 0:nkt // 2, 0:HD],
                                cc_out[tq:2 * tq, h * HD:(h + 1) * HD]
                                .rearrange("(k p) e -> p k e", p=P).bitcast(f32r))
                            nc.sync.dma_start(
                                vt[:, nkt // 2:nkt, 0:HD],
                                cc_out[2 * tq + tq:2 * tk, h * HD:(h + 1) * HD]
                                .rearrange("(k p) e -> p k e", p=P).bitcast(f32r))
                            nc.sync.dma_start(vt[:, :, HD:HD + 1],
                                              ones[:].bitcast(f32r))

                            ot = pot.tile([P, qch], f32)
                            q_rhs = q_all[hh * HD:(hh + 1) * HD, hp,
                                          qc * qch:(qc + 1) * qch]
                            for (k0, g) in kgroups:
                                st = pst.tile([P, 3 * qch], f32, tag="st")
                                for j in range(g):
                                    nc.tensor.matmul(
                                        st[:, j * qch:(j + 1) * qch],
                                        kt[hh * HD:(hh + 1) * HD,
                                               (k0 + j) * P:(k0 + j + 1) * P],
                                        q_rhs,
                                        start=True, stop=True)
                                pt = ptile.tile([P, 3 * qch], f32r, tag="pt")
                                nc.scalar.activation(pt[:, 0:g * qch],
                                                     st[:, 0:g * qch],
                                                     AF.Exp, scale=SCALE)
                                for j in range(g):
                                    nc.tensor.matmul(
                                        ot[0:HD + 1, :],
                                        vt[:, k0 + j, :],
                                        pt[:, j * qch:(j + 1) * qch],
                                        start=(k0 + j == 0),
                                        stop=(k0 + j == nkt - 1))
                            # normalize: 1/Z from PSUM row HD, broadcast to
                            # HD partitions, one fused evict-and-scale multiply
                            # (reciprocal_approx_fast mis-reads PSUM on HW;
                            # stage the Z row through SBUF first)
                            zrow = zpool.tile([1, qch], f32, tag="z")
                            nc.vector.tensor_copy(zrow[:], ot[HD:HD + 1, :])
                            rz = zpool.tile([1, qch], f32, tag="z2")
                            nc.vector.reciprocal_approx_fast(rz[:], zrow[:])
                            rzb = rzbp.tile([HD, qch], f32)
                            nc.gpsimd.partition_broadcast(rzb[:], rz[:])
                            nc.vector.tensor_mul(
                                ot_all[hh * HD:(hh + 1) * HD, hp,
                                       qc * qch:(qc + 1) * qch],
                                ot[0:HD, :], rzb[:])

                    # out projection for this q chunk
                    for jt in range(nft):
                        ps = psy.tile([P, qch], f32)
                        for dc in range(nft):
                            nc.tensor.matmul(
                                ps[:],
                                wp_sb[:, dc, jt * P:(jt + 1) * P],
                                ot_all[:, dc, qc * qch:(qc + 1) * qch],
                                start=(dc == 0), stop=(dc == nft - 1))
                        ysb = ypool.tile([P, qch], f32)
                        nc.vector.tensor_scalar_add(ysb[:], ps[:], bp_sb[:, jt:jt + 1])
                        nc.sync.dma_start(
                            outT[jt * P:(jt + 1) * P, qc * qch:(qc + 1) * qch],
                            ysb[:])

    nc.compile()
    return nc


def make_in_maps(inputs, cfg=None):
    """Host-side sharding: full inputs -> per-core input dicts."""
    if cfg is None:
        cfg = _full_cfg()
    d = cfg["D"]; tq = cfg["TQ"]; n_dev = cfg["n_devices"]
    nft = d // P

    x = np.asarray(inputs["x"], dtype=np.float32)
    w_qkv = np.asarray(inputs["w_qkv"], dtype=np.float32)
    b_qkv = np.asarray(inputs["b_qkv"], dtype=np.float32)
    w_proj = np.asarray(inputs["w_proj"], dtype=np.float32)
    b_proj = np.asarray(inputs["b_proj"], dtype=np.float32)

    x_flat = x.reshape(-1, d)
    w_qT = np.ascontiguousarray(w_qkv[0:d].T)
    w_kT = np.ascontiguousarray(w_qkv[d:2 * d].T)
    w_vT = np.ascontiguousarray(w_qkv[2 * d:3 * d].T)
    b_q = b_qkv[0:d]; b_k = b_qkv[d:2 * d]; b_v = b_qkv[2 * d:3 * d]
    w_pT = np.ascontiguousarray(w_proj.T)
    b_p_eff = b_proj + w_proj @ b_v

    def bias_tile(b):
        return np.ascontiguousarray(b.reshape(nft, P).T)

    nkt = 2 * tq // P
    shared = {
        "ones": np.ones((P, nkt, 1), dtype=np.float32),
        "w_qT": w_qT, "w_kT": w_kT, "w_vT": w_vT, "w_pT": w_pT,
        "b_q": bias_tile(b_q), "b_k": bias_tile(b_k), "b_p": bias_tile(b_p_eff),
    }
    in_maps = []
    for i in range(n_dev):
        xT_i = np.ascontiguousarray(x_flat[i * tq:(i + 1) * tq].T)
        in_maps.append({"xT": xT_i, **shared})
    return in_maps


def assemble_output(results, inputs, cfg=None):
    if cfg is None:
        cfg = _full_cfg()
    d = cfg["D"]; tq = cfg["TQ"]; n_dev = cfg["n_devices"]
    x = np.asarray(inputs["x"])
    y = np.empty((n_dev * tq, d), dtype=np.float32)
    for i in range(n_dev):
        y[i * tq:(i + 1) * tq] = results[i]["out"].T
    return y.reshape(x.shape)


def run(inputs, trace=False, **kw):
    from concourse.bass_utils import run_bass_kernel_spmd
    key = "full"
    if key not in _COMPILED:
        _COMPILED[key] = build()
    nc = _COMPILED[key]
    in_maps = make_in_maps(inputs)
    res = run_bass_kernel_spmd(nc, in_maps, core_ids=list(range(NCORES)),
                               trace=trace, **kw)
    return res


def kernel(**inputs) -> np.ndarray:
    res = run(inputs, trace=False)
    return assemble_output(res.results, inputs)
